# revision 1
# baseline (speedup 1.0000x reference)
"""Expert-parallel MoE MLP (top-2 of 8 experts) on 8 TRN2 NeuronCores.

Strategy (expert-parallel, per sharding hint):
  - core e holds expert e's weights (w1[e], w2[e], host-pre-transposed, bf16)
  - host dispatches tokens by expert id; compute runs over a COMPACT column
    set (ctok = max tokens per expert, padded to 128) instead of the padded
    A2A block layout -- ~20% less matmul work than computing padding
  - mm1 (chunks aligned to A2A trigger tiles) and mm2 (128-token tiles)
    interleaved; each mm2 tile's [128, 1024] result is scaled by the combine
    weights and indirect-DMA-scattered into block-padded per-chunk send
    buffers (relative row indices from the host; rows not in that chunk for
    a given core land on the chunk's garbage row)
  - per-chunk send tensors keep the scatter APs offset-0 AND avoid the
    write-after-read hazard of later scatters against an in-flight A2A read
  - the A2A is split into 3 slot-range chunks on 16-slot boundaries (segment
    sizes multiple of 32KB: measured ~2x collective bandwidth vs unaligned);
    chunk g fires once every core has scattered its chunk-g rows (trigger
    tile T_g baked per run), overlapping the A2A chain with compute
  - a tiny warmup AllToAll at program start absorbs the first-op ring-arming
    latency (~11us otherwise paid on the first real A2A)
  - combine: owner tokens are host-sorted by the highest A2A chunk their
    partial rows land in, so early combine tiles (64 tokens) overlap later
    A2A chunks; single-expert tokens gather the same row twice with a halved
    combine weight (no zero row needed); host unpermutes the output rows
"""

import sys

sys.path.insert(0, "/opt/trn_rl_repo")

import numpy as np
import ml_dtypes

import concourse.bass as bass
import concourse.tile as tile
from concourse import bacc, mybir
from concourse.bass_utils import run_bass_kernel_spmd
from concourse.tile_rust import add_dep_helper

S, DM, DF, E, TOPK = 4096, 1024, 2048, 8, 2
NCORES = 8
P = 128
OWN = S // NCORES  # tokens per owner core
CB = 64  # combine tile rows
F1, F2 = 0.44, 0.70  # chunk slot-bound fractions (rounded to 16)
WARMUP_A2A = True
ACT = mybir.ActivationFunctionType.Silu

_PROGRAM_CACHE: dict = {}


def _split512(n: int) -> list[tuple[int, int]]:
    """Split n cols into (start, size) pieces, multiples of 128, <= 512,
    near-equal so ldweights stays hidden under the moving dim."""
    n_ch = -(-n // 512)
    base = n // n_ch // P * P
    sizes = [base] * n_ch
    rem = n - base * n_ch
    i = 0
    while rem > 0:
        sizes[i] += P
        rem -= P
        i = (i + 1) % n_ch
    out, pos = [], 0
    for s in sizes:
        out.append((pos, s))
        pos += s
    assert pos == n
    return out


def _emit(nc, tc, ctx, cfg):
    blk, B, ctok, Ts, comb_cls, gsets = cfg[:6]
    dt = mybir.dt
    nch = len(B) - 1
    ntiles = ctok // P
    sg = [B[g + 1] - B[g] for g in range(nch)]
    Rg = [NCORES * b for b in B]  # chunk row starts in recvbuf

    xT = nc.dram_tensor("xT", [DM, ctok], dt.bfloat16, kind="ExternalInput").ap()
    w1t = nc.dram_tensor("w1t", [DM, DF], dt.bfloat16, kind="ExternalInput").ap()
    w2t = nc.dram_tensor("w2t", [DF, DM], dt.bfloat16, kind="ExternalInput").ap()
    cv = nc.dram_tensor("cv", [ctok], dt.float32, kind="ExternalInput").ap()
    rscs = [
        nc.dram_tensor(f"rsc{g}", [ctok], dt.int32, kind="ExternalInput").ap()
        for g in range(nch)
    ]
    g0 = nc.dram_tensor("g0", [OWN], dt.int32, kind="ExternalInput").ap()
    g1 = nc.dram_tensor("g1", [OWN], dt.int32, kind="ExternalInput").ap()
    yout = nc.dram_tensor("yout", [OWN, DM], dt.float32, kind="ExternalOutput").ap()
    # per-chunk send tensors, each with +1 garbage row (scatter APs must be
    # offset-0; separate tensors keep chunk deps disjoint)
    sends = [
        nc.dram_tensor(f"send{g}", [NCORES * sg[g] + 1, DM], dt.bfloat16).ap()
        for g in range(nch)
    ]
    recvbuf = nc.dram_tensor("recvbuf", [NCORES * blk, DM], dt.bfloat16).ap()

    wpool = ctx.enter_context(tc.tile_pool(name="w", bufs=1))
    hpool = ctx.enter_context(tc.tile_pool(name="h", bufs=2))
    ypool = ctx.enter_context(tc.tile_pool(name="y", bufs=8))
    gpool = ctx.enter_context(tc.tile_pool(name="g", bufs=3))
    phpool = ctx.enter_context(tc.tile_pool(name="ph", bufs=3, space="PSUM"))
    pypool = ctx.enter_context(tc.tile_pool(name="py", bufs=5, space="PSUM"))

    groups = [list(range(NCORES))]

    if WARMUP_A2A:
        dummy_s = nc.dram_tensor("dummy_s", [NCORES, 32], dt.bfloat16).ap()
        dummy_r = nc.dram_tensor("dummy_r", [NCORES, 32], dt.bfloat16).ap()
        zrow = wpool.tile([NCORES, 32], dt.bfloat16, tag="zrow")
        nc.vector.memset(zrow[:], 0.0)
        nc.sync.dma_start(dummy_s[:, :], zrow[:])
        nc.gpsimd.collective_compute(
            "AllToAll",
            mybir.AluOpType.bypass,
            replica_groups=groups,
            ins=[dummy_s],
            outs=[dummy_r],
        )

    # ---- loads: w1/x(first cols) interleaved (first matmuls need them) ----
    segs = []  # (tile_a, tile_b) mm1 segments aligned to trigger tiles
    prev = 0
    for t in sorted(set(Ts)):
        if t > prev:
            segs.append((prev, t))
            prev = t
    w1sb = wpool.tile([P, DM // P, DF], dt.bfloat16, tag="w1sb")
    w1r = w1t.rearrange("(o p) f -> p o f", p=P)
    xsb = wpool.tile([P, DM // P, ctok], dt.bfloat16, tag="xsb")
    xr = xT.rearrange("(o p) t -> p o t", p=P)
    a0, b0 = segs[0]
    # f-major halves: the first 8 f-tiles' weights + first-segment x land
    # first, so ph_0..7 start ~8us earlier than a k-major load order allows
    for k in range(DM // P):
        nc.sync.dma_start(w1sb[:, k, 0 : DF // 2], w1r[:, k, 0 : DF // 2])
        nc.sync.dma_start(xsb[:, k, a0 * P : b0 * P], xr[:, k, a0 * P : b0 * P])
    for k in range(DM // P):
        nc.sync.dma_start(w1sb[:, k, DF // 2 : DF], w1r[:, k, DF // 2 : DF])
    csb = wpool.tile([P, ntiles], dt.float32, tag="csb")
    nc.sync.dma_start(csb[:], cv.rearrange("(t p) -> p t", p=P))
    rssbs = []
    for g in range(nch):
        rssb = wpool.tile([P, ntiles], dt.int32, tag=f"rssb{g}", name=f"rssb{g}")
        nc.sync.dma_start(rssb[:], rscs[g].rearrange("(t p) -> p t", p=P))
        rssbs.append(rssb)
    for a1, b1 in segs[1:]:
        for k in range(DM // P):
            nc.sync.dma_start(xsb[:, k, a1 * P : b1 * P], xr[:, k, a1 * P : b1 * P])
    w2sb = wpool.tile([P, DF // P, DM], dt.bfloat16, tag="w2sb")
    w2r = w2t.rearrange("(o p) d -> p o d", p=P)
    for f in range(DF // P):
        nc.sync.dma_start(w2sb[:, f, :], w2r[:, f, :])
    g0sb = wpool.tile([CB, OWN // CB], dt.int32, tag="g0sb")
    nc.sync.dma_start(g0sb[:], g0.rearrange("(t p) -> p t", p=CB))
    g1sb = wpool.tile([CB, OWN // CB], dt.int32, tag="g1sb")
    nc.sync.dma_start(g1sb[:], g1.rearrange("(t p) -> p t", p=CB))

    # zero send buffers: padding rows inside each (expert->owner) block are
    # never scattered to, but the A2A ships them. After the loads so the
    # first matmuls aren't starved.
    zt = wpool.tile([P, DM], dt.bfloat16, tag="zt")
    nc.vector.memset(zt[:], 0.0)
    for g in range(nch):
        nrows = NCORES * sg[g] + 1
        for r0 in range(0, nrows, P):
            rn = min(P, nrows - r0)
            nc.sync.dma_start(sends[g][r0 : r0 + rn, :], zt[0:rn, :])

    # ---- interleaved mm1 (trigger-aligned chunks) / mm2 (token tiles) ----
    fired = [False] * nch
    scat_gate = [None] * nch
    hs = None
    hbase = 0
    done_mm1 = 0
    si = 0

    def emit_mm1():
        nonlocal hs, hbase, done_mm1, si
        a, b = segs[si]
        si += 1
        c0, csz = a * P, (b - a) * P
        hbase = c0
        done_mm1 = c0 + csz
        hs = [
            hpool.tile([P, csz], dt.bfloat16, tag=f"h{i}", name=f"h{i}")
            for i in range(DF // P)
        ]
        for i in range(DF // P):
            for s0, ssz in _split512(csz):
                ph = phpool.tile([P, ssz], dt.float32, tag="ph")
                for k in range(DM // P):
                    nc.tensor.matmul(
                        ph[:],
                        lhsT=w1sb[:, k, i * P : (i + 1) * P],
                        rhs=xsb[:, k, c0 + s0 : c0 + s0 + ssz],
                        start=(k == 0),
                        stop=(k == DM // P - 1),
                    )
                nc.scalar.activation(hs[i][:, s0 : s0 + ssz], ph[:], ACT)

    for tm in range(ntiles):
        while (tm + 1) * P > done_mm1:
            emit_mm1()
        toff = tm * P - hbase
        py0 = pypool.tile([P, 512], dt.float32, tag="py")
        py1 = pypool.tile([P, 512], dt.float32, tag="py")
        for f in range(DF // P):
            lhs = hs[f][:, toff : toff + P]
            nc.tensor.matmul(
                py0[:], lhsT=lhs, rhs=w2sb[:, f, 0:512],
                start=(f == 0), stop=(f == DF // P - 1),
            )
            nc.tensor.matmul(
                py1[:], lhsT=lhs, rhs=w2sb[:, f, 512:1024],
                start=(f == 0), stop=(f == DF // P - 1),
            )
        # scale on the (otherwise idle) Scalar engine: keeping these off the
        # Vector queue stops the combine adds' gather-waits from head-blocking
        # the PSUM release chain
        y_sb = ypool.tile([P, DM], dt.bfloat16, tag="y")
        nc.scalar.activation(
            y_sb[:, 0:512], py0[:],
            mybir.ActivationFunctionType.Copy, scale=csb[:, tm : tm + 1],
        )
        nc.scalar.activation(
            y_sb[:, 512:1024], py1[:],
            mybir.ActivationFunctionType.Copy, scale=csb[:, tm : tm + 1],
        )
        for g in gsets[tm]:
            last_scat = nc.gpsimd.indirect_dma_start(
                out=sends[g][:],
                out_offset=bass.IndirectOffsetOnAxis(
                    ap=rssbs[g][:, tm : tm + 1], axis=0
                ),
                in_=y_sb[:],
                in_offset=None,
            )
        for g in range(nch):
            if not fired[g] and Ts[g] == tm + 1:
                nc.gpsimd.collective_compute(
                    "AllToAll",
                    mybir.AluOpType.bypass,
                    replica_groups=groups,
                    ins=[sends[g][0 : NCORES * sg[g], :]],
                    outs=[recvbuf[Rg[g] : Rg[g + 1], :]],
                )
                fired[g] = True
                scat_gate[g] = last_scat
    assert all(fired), (Ts, ntiles)

    # ---- combine: class-sorted 64-token tiles, prefix-sliced gathers ----
    for j in range(OWN // CB):
        pref = Rg[comb_cls[j] + 1]
        ga = gpool.tile([CB, DM], dt.bfloat16, tag="ga")
        gi0 = nc.gpsimd.indirect_dma_start(
            out=ga[:],
            out_offset=None,
            in_=recvbuf[0:pref, :],
            in_offset=bass.IndirectOffsetOnAxis(ap=g0sb[:, j : j + 1], axis=0),
        )
        gb = gpool.tile([CB, DM], dt.bfloat16, tag="gb")
        gi1 = nc.gpsimd.indirect_dma_start(
            out=gb[:],
            out_offset=None,
            in_=recvbuf[0:pref, :],
            in_offset=bass.IndirectOffsetOnAxis(ap=g1sb[:, j : j + 1], axis=0),
        )
        # queue class-c gathers behind the scatter that fires chunk c+1: the
        # scheduler's optimistic A2A cost model otherwise slots them between
        # scatters where their (blocked) recvbuf wait head-blocks the gpsimd
        # queue and delays the next A2A trigger by ~7us. Gating on c+1's
        # scatter (not the overall last) still lets early-class gathers
        # overlap the later A2A chunks.
        c = comb_cls[j]
        gate = scat_gate[c + 1] if c + 1 < nch else scat_gate[nch - 1]
        add_dep_helper(gi0.ins, gate.ins, sync=False, reason="scatters first")
        add_dep_helper(gi1.ins, gate.ins, sync=False, reason="scatters first")
        ys = gpool.tile([CB, DM], dt.float32, tag="ys")
        nc.vector.tensor_add(ys[:], ga[:], gb[:])
        nc.sync.dma_start(yout[j * CB : (j + 1) * CB, :], ys[:])


def _build_program(cfg):
    key = cfg[:6]
    if key in _PROGRAM_CACHE:
        return _PROGRAM_CACHE[key]
    from contextlib import ExitStack

    nc = bacc.Bacc(
        "TRN2",
        target_bir_lowering=False,
        debug=False,
        enable_asserts=True,
        num_devices=NCORES,
    )
    with tile.TileContext(nc) as tc:
        with ExitStack() as ctx:
            _emit(nc, tc, ctx, cfg)
    nc.compile()
    _PROGRAM_CACHE[key] = nc
    return nc


def _prepare(x, topk_e, topk_w):
    """Host-side routing: dispatch tokens to experts.

    Layout: token with slot s in (expert e -> owner d) block, s in
    [B_g, B_{g+1}):  send-side row (core e, tensor send_g) = d*sg + (s-B_g);
    recv-side row (core d, recvbuf) = 8*B_g + e*sg + (s-B_g).
    """
    bf16 = ml_dtypes.bfloat16
    c = np.zeros((S, E), dtype=np.float32)
    np.add.at(c, (np.arange(S)[:, None], topk_e), topk_w.astype(np.float32))
    single = topk_e[:, 0] == topk_e[:, 1]

    toks = [np.nonzero((topk_e == e).any(axis=1))[0] for e in range(E)]
    cnt = np.zeros((E, NCORES), dtype=np.int64)
    for e in range(E):
        cnt[e] = np.bincount(toks[e] // OWN, minlength=NCORES)
    blk = int(cnt.max())
    b1 = min(blk - 32, max(16, int(round(blk * F1 / 16)) * 16))
    b2 = min(blk - 16, max(b1 + 16, int(round(blk * F2 / 16)) * 16))
    B = sorted(set([0, b1, b2, blk]))
    nch = len(B) - 1
    Ba = np.array(B)
    sga = np.diff(Ba)
    Rga = NCORES * Ba

    ntok_max = max(len(t) for t in toks)
    ctok = int(-(-ntok_max // P) * P)
    ntiles = ctok // P

    in_maps = []
    row_of = {}  # (e, token) -> absolute recvbuf row (on the owner core)
    cums = np.zeros((E, nch), dtype=np.int64)
    gsets = [set() for _ in range(ntiles)]
    for e in range(E):
        te = toks[e]
        d = te // OWN
        seg_start = np.searchsorted(te, np.arange(NCORES) * OWN)
        slot = np.arange(len(te)) - seg_start[d]
        gi = np.searchsorted(Ba[1:-1], slot, side="right")
        srow = d * sga[gi] + (slot - Ba[gi])  # send side, relative to send_g
        rrow = Rga[gi] + e * sga[gi] + (slot - Ba[gi])  # recv side, absolute
        for t, r in zip(te, rrow):
            row_of[(e, int(t))] = int(r)
        order = np.lexsort((slot, d, gi))
        te_o = te[order]
        gi_o = np.full(ctok, nch - 1, dtype=np.int64)
        gi_o[: len(te)] = gi[order]
        xT_e = np.zeros((DM, ctok), dtype=bf16)
        xT_e[:, : len(te)] = x[te_o].T.astype(bf16)
        cv_e = np.zeros(ctok, dtype=np.float32)
        w = c[te_o, e]
        cv_e[: len(te)] = np.where(single[te_o], 0.5 * w, w)
        im = {"xT": xT_e, "cv": cv_e}
        srow_o = np.zeros(ctok, dtype=np.int64)
        srow_o[: len(te)] = srow[order]
        for g in range(nch):
            rs = np.full(ctok, NCORES * sga[g], dtype=np.int32)  # garbage row
            sel = gi_o == g
            sel[len(te) :] = False
            rs[sel] = srow_o[sel]
            im[f"rsc{g}"] = rs
            cums[e, g] = int(np.sum(gi <= g))
        for tm in range(ntiles):
            for g in np.unique(gi_o[tm * P : (tm + 1) * P]):
                gsets[tm].add(int(g))
        in_maps.append(im)

    Ts = [min(int(np.ceil(cums[:, g].max() / P)), ntiles) for g in range(nch)]
    for g in range(1, nch):
        Ts[g] = max(Ts[g], Ts[g - 1])
    Ts[-1] = ntiles

    chunk_of_row = lambda r: int(np.searchsorted(Rga[1:], r, side="right"))
    perms = []
    comb_cls = np.zeros((NCORES, OWN // CB), dtype=np.int64)
    for dcore in range(NCORES):
        r0a = np.zeros(OWN, dtype=np.int32)
        r1a = np.zeros(OWN, dtype=np.int32)
        cls = np.zeros(OWN, dtype=np.int64)
        for t_loc in range(OWN):
            t = dcore * OWN + t_loc
            es = np.unique(topk_e[t])
            ra = row_of[(int(es[0]), t)]
            rb = row_of[(int(es[1]), t)] if len(es) > 1 else ra
            r0a[t_loc], r1a[t_loc] = ra, rb
            cls[t_loc] = max(chunk_of_row(ra), chunk_of_row(rb))
        perm = np.argsort(cls, kind="stable")
        perms.append(perm)
        comb_cls[dcore] = cls[perm].reshape(OWN // CB, CB).max(axis=1)
        in_maps[dcore]["g0"] = r0a[perm]
        in_maps[dcore]["g1"] = r1a[perm]

    cfg = (
        blk,
        tuple(B),
        ctok,
        tuple(Ts),
        tuple(int(v) for v in comb_cls.max(axis=0)),
        tuple(tuple(sorted(s)) for s in gsets),
        tuple(tuple(int(v) for v in p) for p in perms),
    )
    return in_maps, cfg


def prepare_in_maps(x, topk_e, topk_w, w1, w2):
    bf16 = ml_dtypes.bfloat16
    in_maps, cfg = _prepare(np.asarray(x), np.asarray(topk_e), np.asarray(topk_w))
    for e in range(E):
        in_maps[e]["w1t"] = np.ascontiguousarray(np.asarray(w1)[e].T).astype(bf16)
        in_maps[e]["w2t"] = np.ascontiguousarray(np.asarray(w2)[e].T).astype(bf16)
    return in_maps, cfg


def postprocess(results, cfg):
    perms = cfg[6]
    out = np.empty((S, DM), dtype=np.float32)
    for d in range(NCORES):
        out[d * OWN + np.asarray(perms[d], dtype=np.int64)] = results[d]["yout"]
    return out


def kernel(x, topk_e, topk_w, w1, w2):
    in_maps, cfg = prepare_in_maps(x, topk_e, topk_w, w1, w2)
    nc = _build_program(cfg)
    res = run_bass_kernel_spmd(nc, in_maps, list(range(NCORES)))
    return postprocess(res.results, cfg)



# revision 3
# speedup vs baseline: 44275.4851x; 44275.4851x over previous
"""Expert-parallel MoE MLP (top-2 of 8 experts) on 8 TRN2 NeuronCores.

Strategy (expert-parallel, per sharding hint):
  - core e holds expert e's weights (w1[e], w2[e], host-pre-transposed, bf16)
  - host dispatches tokens by expert id; compute runs over a COMPACT column
    set (ctok = max tokens per expert, padded to 128)
  - mm1 emitted in 2-tile (256-col) segments interleaved with mm2 128-token
    tiles; each mm2 tile's [128, 1024] result is scaled by the combine
    weights and indirect-DMA-scattered into block-padded per-chunk send
    buffers (relative row indices from the host)
  - the A2A is split into up to 4 slot-range chunks on 16-slot boundaries,
    sized adaptively so they trigger after tiles [n/2, n-2, n-1, n]: the
    last chunk is small (only the deepest slots) so the post-compute tail
    is one short collective instead of a third of the payload
  - a tiny warmup AllToAll at program start absorbs the ~60us first-op
    ring-arming barrier; real chunks then run at wire speed (~3.5us + 9us/MB)
  - input loads are issued in parallel across the SP/Activation/DVE queues
    with few, large DMAs (multi-k descriptors) so the first matmuls start
    ~5us after the preamble instead of serializing behind one issue queue
  - send-buffer padding rows are never zeroed: the A2A ships garbage in
    them but the combine gathers only reference real rows
  - combine: owner tokens are host-sorted by the highest A2A chunk their
    partial rows land in, so early combine tiles (64 tokens) overlap later
    A2A chunks; single-expert tokens gather the same row twice with a
    halved combine weight; host unpermutes the output rows
"""

import sys

sys.path.insert(0, "/opt/trn_rl_repo")

import numpy as np
import ml_dtypes

import concourse.bass as bass
import concourse.tile as tile
from concourse import bacc, mybir
from concourse.bass_utils import run_bass_kernel_spmd
from concourse.tile_rust import add_dep_helper

S, DM, DF, E, TOPK = 4096, 1024, 2048, 8, 2
NCORES = 8
P = 128
OWN = S // NCORES  # tokens per owner core
CB = 64  # combine tile rows
WARMUP_A2A = True
ACT = mybir.ActivationFunctionType.Silu

_PROGRAM_CACHE: dict = {}


def _emit(nc, tc, ctx, cfg):
    blk, B, ctok, Ts, comb_cls, gsets = cfg[:6]
    dt = mybir.dt
    nch = len(B) - 1
    ntiles = ctok // P
    sg = [B[g + 1] - B[g] for g in range(nch)]
    Rg = [NCORES * b for b in B]  # chunk row starts in recvbuf

    xT = nc.dram_tensor("xT", [DM, ctok], dt.bfloat16, kind="ExternalInput").ap()
    w1t = nc.dram_tensor("w1t", [DM, DF], dt.bfloat16, kind="ExternalInput").ap()
    w2t = nc.dram_tensor("w2t", [DF, DM], dt.bfloat16, kind="ExternalInput").ap()
    cv = nc.dram_tensor("cv", [ctok], dt.float32, kind="ExternalInput").ap()
    rscs = [
        nc.dram_tensor(f"rsc{g}", [ctok], dt.int32, kind="ExternalInput").ap()
        for g in range(nch)
    ]
    g0 = nc.dram_tensor("g0", [OWN], dt.int32, kind="ExternalInput").ap()
    g1 = nc.dram_tensor("g1", [OWN], dt.int32, kind="ExternalInput").ap()
    yout = nc.dram_tensor("yout", [OWN, DM], dt.float32, kind="ExternalOutput").ap()
    # per-chunk send tensors, each with +1 garbage row (scatter APs must be
    # offset-0; separate tensors keep chunk deps disjoint)
    sends = [
        nc.dram_tensor(f"send{g}", [NCORES * sg[g] + 1, DM], dt.bfloat16).ap()
        for g in range(nch)
    ]
    recvbuf = nc.dram_tensor("recvbuf", [NCORES * blk, DM], dt.bfloat16).ap()

    wpool = ctx.enter_context(tc.tile_pool(name="w", bufs=1))
    hpool = ctx.enter_context(tc.tile_pool(name="h", bufs=2))
    ypool = ctx.enter_context(tc.tile_pool(name="y", bufs=8))
    gpool = ctx.enter_context(tc.tile_pool(name="g", bufs=3))
    phpool = ctx.enter_context(tc.tile_pool(name="ph", bufs=3, space="PSUM"))
    pypool = ctx.enter_context(tc.tile_pool(name="py", bufs=5, space="PSUM"))

    groups = [list(range(NCORES))]

    if WARMUP_A2A:
        dummy_s = nc.dram_tensor("dummy_s", [NCORES, 32], dt.bfloat16).ap()
        dummy_r = nc.dram_tensor("dummy_r", [NCORES, 32], dt.bfloat16).ap()
        zrow = wpool.tile([NCORES, 32], dt.bfloat16, tag="zrow")
        nc.vector.memset(zrow[:], 0.0)
        nc.gpsimd.dma_start(dummy_s[:, :], zrow[:])
        nc.gpsimd.collective_compute(
            "AllToAll",
            mybir.AluOpType.bypass,
            replica_groups=groups,
            ins=[dummy_s],
            outs=[dummy_r],
        )

    # ---- loads: few large DMAs spread across 3 issue queues -------------
    # sync (SP): x first-half then second-half, in 4-k groups (first mm1
    # segment needs all k of its columns; 4-k granularity lets the k<4
    # matmuls start one transfer earlier)
    # scalar:    w1 (first f-half first: first 8 f-tiles' ph come first)
    # vector:    combine-weight/scatter-row/gather-idx smalls, then w2
    w1sb = wpool.tile([P, DM // P, DF], dt.bfloat16, tag="w1sb")
    w1r = w1t.rearrange("(o p) f -> p o f", p=P)
    xsb = wpool.tile([P, DM // P, ctok], dt.bfloat16, tag="xsb")
    xr = xT.rearrange("(o p) t -> p o t", p=P)
    half = (ctok // 2) // P * P
    for k0 in range(0, DM // P, 4):
        nc.sync.dma_start(xsb[:, k0 : k0 + 4, 0:half], xr[:, k0 : k0 + 4, 0:half])
    for k0 in range(0, DM // P, 4):
        nc.scalar.dma_start(
            w1sb[:, k0 : k0 + 4, 0 : DF // 2], w1r[:, k0 : k0 + 4, 0 : DF // 2]
        )
    for k0 in range(0, DM // P, 4):
        nc.sync.dma_start(
            xsb[:, k0 : k0 + 4, half:ctok], xr[:, k0 : k0 + 4, half:ctok]
        )
    for k0 in range(0, DM // P, 4):
        nc.scalar.dma_start(
            w1sb[:, k0 : k0 + 4, DF // 2 : DF], w1r[:, k0 : k0 + 4, DF // 2 : DF]
        )
    csb = wpool.tile([P, ntiles], dt.float32, tag="csb")
    nc.gpsimd.dma_start(csb[:], cv.rearrange("(t p) -> p t", p=P))
    rssbs = []
    for g in range(nch):
        rssb = wpool.tile([P, ntiles], dt.int32, tag=f"rssb{g}", name=f"rssb{g}")
        nc.gpsimd.dma_start(rssb[:], rscs[g].rearrange("(t p) -> p t", p=P))
        rssbs.append(rssb)
    g0sb = wpool.tile([CB, OWN // CB], dt.int32, tag="g0sb")
    nc.gpsimd.dma_start(g0sb[:], g0.rearrange("(t p) -> p t", p=CB))
    g1sb = wpool.tile([CB, OWN // CB], dt.int32, tag="g1sb")
    nc.gpsimd.dma_start(g1sb[:], g1.rearrange("(t p) -> p t", p=CB))
    w2sb = wpool.tile([P, DF // P, DM], dt.bfloat16, tag="w2sb")
    w2r = w2t.rearrange("(o p) d -> p o d", p=P)
    for f0 in range(0, DF // P, 4):
        nc.scalar.dma_start(w2sb[:, f0 : f0 + 4, :], w2r[:, f0 : f0 + 4, :])

    # ---- interleaved mm1 (2-tile segments) / mm2 (token tiles) ----------
    # mm1 segments are 256 cols: wide enough that ldweights stays hidden
    # under the moving dim, narrow enough that mm2 tiles start early
    segs = []
    t = 0
    while t < ntiles:
        b = min(t + 2, ntiles)
        segs.append((t, b))
        t = b

    fired = [False] * nch
    scat_gate = [None] * nch
    hs = None
    hbase = 0
    done_mm1 = 0
    si = 0

    def emit_mm1():
        nonlocal hs, hbase, done_mm1, si
        a, b = segs[si]
        si += 1
        c0, csz = a * P, (b - a) * P
        hbase = c0
        done_mm1 = c0 + csz
        hs = [
            hpool.tile([P, csz], dt.bfloat16, tag=f"h{i}", name=f"h{i}")
            for i in range(DF // P)
        ]
        for i in range(DF // P):
            ph = phpool.tile([P, csz], dt.float32, tag="ph")
            for k in range(DM // P):
                nc.tensor.matmul(
                    ph[:],
                    lhsT=w1sb[:, k, i * P : (i + 1) * P],
                    rhs=xsb[:, k, c0 : c0 + csz],
                    start=(k == 0),
                    stop=(k == DM // P - 1),
                )
            nc.scalar.activation(hs[i][:], ph[:], ACT)

    for tm in range(ntiles):
        while (tm + 1) * P > done_mm1:
            emit_mm1()
        toff = tm * P - hbase
        py0 = pypool.tile([P, 512], dt.float32, tag="py")
        py1 = pypool.tile([P, 512], dt.float32, tag="py")
        for f in range(DF // P):
            lhs = hs[f][:, toff : toff + P]
            nc.tensor.matmul(
                py0[:], lhsT=lhs, rhs=w2sb[:, f, 0:512],
                start=(f == 0), stop=(f == DF // P - 1),
            )
            nc.tensor.matmul(
                py1[:], lhsT=lhs, rhs=w2sb[:, f, 512:1024],
                start=(f == 0), stop=(f == DF // P - 1),
            )
        # scale on the (otherwise idle) Scalar engine: keeping these off the
        # Vector queue stops the combine adds' gather-waits from head-blocking
        # the PSUM release chain
        y_sb = ypool.tile([P, DM], dt.bfloat16, tag="y")
        nc.scalar.activation(
            y_sb[:, 0:512], py0[:],
            mybir.ActivationFunctionType.Copy, scale=csb[:, tm : tm + 1],
        )
        nc.scalar.activation(
            y_sb[:, 512:1024], py1[:],
            mybir.ActivationFunctionType.Copy, scale=csb[:, tm : tm + 1],
        )
        for g in gsets[tm]:
            last_scat = nc.gpsimd.indirect_dma_start(
                out=sends[g][:],
                out_offset=bass.IndirectOffsetOnAxis(
                    ap=rssbs[g][:, tm : tm + 1], axis=0
                ),
                in_=y_sb[:],
                in_offset=None,
            )
        for g in range(nch):
            if not fired[g] and Ts[g] == tm + 1:
                nc.gpsimd.collective_compute(
                    "AllToAll",
                    mybir.AluOpType.bypass,
                    replica_groups=groups,
                    ins=[sends[g][0 : NCORES * sg[g], :]],
                    outs=[recvbuf[Rg[g] : Rg[g + 1], :]],
                )
                fired[g] = True
                scat_gate[g] = last_scat
    assert all(fired), (Ts, ntiles)

    # ---- combine: class-sorted 64-token tiles, prefix-sliced gathers ----
    for j in range(OWN // CB):
        pref = Rg[comb_cls[j] + 1]
        ga = gpool.tile([CB, DM], dt.bfloat16, tag="ga")
        gi0 = nc.gpsimd.indirect_dma_start(
            out=ga[:],
            out_offset=None,
            in_=recvbuf[0:pref, :],
            in_offset=bass.IndirectOffsetOnAxis(ap=g0sb[:, j : j + 1], axis=0),
        )
        gb = gpool.tile([CB, DM], dt.bfloat16, tag="gb")
        gi1 = nc.gpsimd.indirect_dma_start(
            out=gb[:],
            out_offset=None,
            in_=recvbuf[0:pref, :],
            in_offset=bass.IndirectOffsetOnAxis(ap=g1sb[:, j : j + 1], axis=0),
        )
        # queue class-c gathers behind the scatter that fires chunk c+1: the
        # scheduler's optimistic A2A cost model otherwise slots them between
        # scatters where their (blocked) recvbuf wait head-blocks the gpsimd
        # queue and delays the next A2A trigger. Gating on c+1's scatter
        # (not the overall last) still lets early-class gathers overlap the
        # later A2A chunks.
        c = comb_cls[j]
        gate = scat_gate[c + 1] if c + 1 < nch else scat_gate[nch - 1]
        add_dep_helper(gi0.ins, gate.ins, sync=False, reason="scatters first")
        add_dep_helper(gi1.ins, gate.ins, sync=False, reason="scatters first")
        ys = gpool.tile([CB, DM], dt.float32, tag="ys")
        nc.vector.tensor_add(ys[:], ga[:], gb[:])
        nc.sync.dma_start(yout[j * CB : (j + 1) * CB, :], ys[:])


def _build_program(cfg):
    key = cfg[:6]
    if key in _PROGRAM_CACHE:
        return _PROGRAM_CACHE[key]
    from contextlib import ExitStack

    nc = bacc.Bacc(
        "TRN2",
        target_bir_lowering=False,
        debug=False,
        enable_asserts=True,
        num_devices=NCORES,
    )
    with tile.TileContext(nc) as tc:
        with ExitStack() as ctx:
            _emit(nc, tc, ctx, cfg)
    nc.compile()
    _PROGRAM_CACHE[key] = nc
    return nc


def _prepare(x, topk_e, topk_w):
    """Host-side routing: dispatch tokens to experts.

    Layout: token with slot s in (expert e -> owner d) block, s in
    [B_g, B_{g+1}):  send-side row (core e, tensor send_g) = d*sg + (s-B_g);
    recv-side row (core d, recvbuf) = 8*B_g + e*sg + (s-B_g).
    """
    bf16 = ml_dtypes.bfloat16
    c = np.zeros((S, E), dtype=np.float32)
    np.add.at(c, (np.arange(S)[:, None], topk_e), topk_w.astype(np.float32))
    single = topk_e[:, 0] == topk_e[:, 1]

    toks = [np.nonzero((topk_e == e).any(axis=1))[0] for e in range(E)]
    cnt = np.zeros((E, NCORES), dtype=np.int64)
    for e in range(E):
        cnt[e] = np.bincount(toks[e] // OWN, minlength=NCORES)
    blk = int(cnt.max())

    ntok_max = max(len(t) for t in toks)
    ctok = int(-(-ntok_max // P) * P)
    ntiles = ctok // P

    # Adaptive chunk boundaries: chunk g should trigger after tile
    # targets[g], i.e. the largest 16-multiple slot b such that every
    # expert's token count with slot < b fits in targets[g]*128 rows.
    # Late triggers get small chunks so the post-compute tail is short.
    def maxcum(b):
        return int(np.minimum(cnt, b).sum(axis=1).max())

    targets = [ntiles // 2, ntiles - 2, ntiles - 1]
    B = [0]
    for tgt in targets:
        b = B[-1]
        nb = b
        while nb + 16 < blk and maxcum(nb + 16) <= tgt * P:
            nb += 16
        if nb > b:
            B.append(nb)
    B.append(blk)
    B = sorted(set(B))
    nch = len(B) - 1
    Ba = np.array(B)
    sga = np.diff(Ba)
    Rga = NCORES * Ba

    in_maps = []
    row_of = {}  # (e, token) -> absolute recvbuf row (on the owner core)
    cums = np.zeros((E, nch), dtype=np.int64)
    gsets = [set() for _ in range(ntiles)]
    for e in range(E):
        te = toks[e]
        d = te // OWN
        seg_start = np.searchsorted(te, np.arange(NCORES) * OWN)
        slot = np.arange(len(te)) - seg_start[d]
        gi = np.searchsorted(Ba[1:-1], slot, side="right")
        srow = d * sga[gi] + (slot - Ba[gi])  # send side, relative to send_g
        rrow = Rga[gi] + e * sga[gi] + (slot - Ba[gi])  # recv side, absolute
        for t, r in zip(te, rrow):
            row_of[(e, int(t))] = int(r)
        order = np.lexsort((slot, d, gi))
        te_o = te[order]
        gi_o = np.full(ctok, nch - 1, dtype=np.int64)
        gi_o[: len(te)] = gi[order]
        xT_e = np.zeros((DM, ctok), dtype=bf16)
        xT_e[:, : len(te)] = x[te_o].T.astype(bf16)
        cv_e = np.zeros(ctok, dtype=np.float32)
        w = c[te_o, e]
        cv_e[: len(te)] = np.where(single[te_o], 0.5 * w, w)
        im = {"xT": xT_e, "cv": cv_e}
        srow_o = np.zeros(ctok, dtype=np.int64)
        srow_o[: len(te)] = srow[order]
        for g in range(nch):
            rs = np.full(ctok, NCORES * sga[g], dtype=np.int32)  # garbage row
            sel = gi_o == g
            sel[len(te) :] = False
            rs[sel] = srow_o[sel]
            im[f"rsc{g}"] = rs
            cums[e, g] = int(np.sum(gi <= g))
        for tm in range(ntiles):
            for g in np.unique(gi_o[tm * P : (tm + 1) * P]):
                gsets[tm].add(int(g))
        in_maps.append(im)

    Ts = [min(int(np.ceil(cums[:, g].max() / P)), ntiles) for g in range(nch)]
    for g in range(1, nch):
        Ts[g] = max(Ts[g], Ts[g - 1])
    Ts[-1] = ntiles

    chunk_of_row = lambda r: int(np.searchsorted(Rga[1:], r, side="right"))
    perms = []
    comb_cls = np.zeros((NCORES, OWN // CB), dtype=np.int64)
    for dcore in range(NCORES):
        r0a = np.zeros(OWN, dtype=np.int32)
        r1a = np.zeros(OWN, dtype=np.int32)
        cls = np.zeros(OWN, dtype=np.int64)
        for t_loc in range(OWN):
            t = dcore * OWN + t_loc
            es = np.unique(topk_e[t])
            ra = row_of[(int(es[0]), t)]
            rb = row_of[(int(es[1]), t)] if len(es) > 1 else ra
            r0a[t_loc], r1a[t_loc] = ra, rb
            cls[t_loc] = max(chunk_of_row(ra), chunk_of_row(rb))
        perm = np.argsort(cls, kind="stable")
        perms.append(perm)
        comb_cls[dcore] = cls[perm].reshape(OWN // CB, CB).max(axis=1)
        in_maps[dcore]["g0"] = r0a[perm]
        in_maps[dcore]["g1"] = r1a[perm]

    cfg = (
        blk,
        tuple(B),
        ctok,
        tuple(Ts),
        tuple(int(v) for v in comb_cls.max(axis=0)),
        tuple(tuple(sorted(s)) for s in gsets),
        tuple(tuple(int(v) for v in p) for p in perms),
    )
    return in_maps, cfg


def prepare_in_maps(x, topk_e, topk_w, w1, w2):
    bf16 = ml_dtypes.bfloat16
    in_maps, cfg = _prepare(np.asarray(x), np.asarray(topk_e), np.asarray(topk_w))
    for e in range(E):
        in_maps[e]["w1t"] = np.ascontiguousarray(np.asarray(w1)[e].T).astype(bf16)
        in_maps[e]["w2t"] = np.ascontiguousarray(np.asarray(w2)[e].T).astype(bf16)
    return in_maps, cfg


def postprocess(results, cfg):
    perms = cfg[6]
    out = np.empty((S, DM), dtype=np.float32)
    for d in range(NCORES):
        out[d * OWN + np.asarray(perms[d], dtype=np.int64)] = results[d]["yout"]
    return out


def kernel(x, topk_e, topk_w, w1, w2):
    in_maps, cfg = prepare_in_maps(x, topk_e, topk_w, w1, w2)
    nc = _build_program(cfg)
    res = run_bass_kernel_spmd(nc, in_maps, list(range(NCORES)))
    return postprocess(res.results, cfg)


# revision 9
# speedup vs baseline: 44919.6443x; 1.0145x over previous
"""Expert-parallel MoE MLP (top-2 of 8 experts) on 8 TRN2 NeuronCores.

Strategy (expert-parallel, per sharding hint):
  - core e holds expert e's weights (w1[e], w2[e], host-pre-transposed, bf16)
  - host dispatches tokens by expert id; compute runs over a COMPACT column
    set (ctok = max tokens per expert, padded to 128)
  - mm1 emitted in 2-tile (256-col) segments interleaved with mm2 128-token
    tiles; each mm2 tile's [128, 1024] result is scaled by the combine
    weights and indirect-DMA-scattered into block-padded per-chunk send
    buffers (relative row indices from the host)
  - the A2A is split into up to 4 slot-range chunks on 16-slot boundaries,
    sized adaptively so they trigger after tiles [n/2, n-2, n-1, n]: the
    last chunk is small (only the deepest slots) so the post-compute tail
    is one short collective instead of a third of the payload
  - a tiny warmup AllToAll at program start absorbs the ~60us first-op
    ring-arming barrier; real chunks then run at wire speed (~3.5us + 9us/MB)
  - input loads are issued in parallel across the SP/Activation/DVE queues
    with few, large DMAs (multi-k descriptors) so the first matmuls start
    ~5us after the preamble instead of serializing behind one issue queue
  - send-buffer padding rows are never zeroed: the A2A ships garbage in
    them but the combine gathers only reference real rows
  - combine: owner tokens are host-sorted by the highest A2A chunk their
    partial rows land in, so early combine tiles (64 tokens) overlap later
    A2A chunks; single-expert tokens gather the same row twice with a
    halved combine weight; host unpermutes the output rows
"""

import sys

sys.path.insert(0, "/opt/trn_rl_repo")

import numpy as np
import ml_dtypes

import concourse.bass as bass
import concourse.tile as tile
from concourse import bacc, mybir
from concourse.bass_utils import run_bass_kernel_spmd
from concourse.tile_rust import add_dep_helper

S, DM, DF, E, TOPK = 4096, 1024, 2048, 8, 2
NCORES = 8
P = 128
OWN = S // NCORES  # tokens per owner core
CB = 64  # combine tile rows
WARMUP_A2A = True
ACT = mybir.ActivationFunctionType.Silu

_PROGRAM_CACHE: dict = {}


def _emit(nc, tc, ctx, cfg):
    blk, B, ctok, tr, Ts, comb_cls0, comb_cls1, gsets = cfg[:8]
    dt = mybir.dt
    nch = len(B) - 1
    ntiles = ctok // P
    sg = [B[g + 1] - B[g] for g in range(nch)]
    Rg = [NCORES * b for b in B]  # chunk row starts in recvbuf

    xT = nc.dram_tensor("xT", [DM, ctok], dt.bfloat16, kind="ExternalInput").ap()
    w1t = nc.dram_tensor("w1t", [DM, DF], dt.bfloat16, kind="ExternalInput").ap()
    w2t = nc.dram_tensor("w2t", [DF, DM], dt.bfloat16, kind="ExternalInput").ap()
    cv = nc.dram_tensor("cv", [ctok], dt.float32, kind="ExternalInput").ap()
    rscs = [
        nc.dram_tensor(f"rsc{g}", [ctok], dt.int32, kind="ExternalInput").ap()
        for g in range(nch)
    ]
    g0 = nc.dram_tensor("g0", [OWN], dt.int32, kind="ExternalInput").ap()
    g1 = nc.dram_tensor("g1", [OWN], dt.int32, kind="ExternalInput").ap()
    yout = nc.dram_tensor("yout", [OWN, DM], dt.float32, kind="ExternalOutput").ap()
    # per-chunk send tensors, each with +1 garbage row (scatter APs must be
    # offset-0; separate tensors keep chunk deps disjoint)
    sends = [
        nc.dram_tensor(f"send{g}", [NCORES * sg[g] + 1, DM], dt.bfloat16).ap()
        for g in range(nch)
    ]
    recvbuf = nc.dram_tensor("recvbuf", [NCORES * blk, DM], dt.bfloat16).ap()

    wpool = ctx.enter_context(tc.tile_pool(name="w", bufs=1))
    hpool = ctx.enter_context(tc.tile_pool(name="h", bufs=2))
    ypool = ctx.enter_context(tc.tile_pool(name="y", bufs=8))
    gpool = ctx.enter_context(tc.tile_pool(name="g", bufs=3))
    phpool = ctx.enter_context(tc.tile_pool(name="ph", bufs=3, space="PSUM"))
    pypool = ctx.enter_context(tc.tile_pool(name="py", bufs=5, space="PSUM"))

    groups = [list(range(NCORES))]

    if WARMUP_A2A:
        dummy_s = nc.dram_tensor("dummy_s", [NCORES, 32], dt.bfloat16).ap()
        dummy_r = nc.dram_tensor("dummy_r", [NCORES, 32], dt.bfloat16).ap()
        zrow = wpool.tile([NCORES, 32], dt.bfloat16, tag="zrow")
        nc.vector.memset(zrow[:], 0.0)
        nc.gpsimd.dma_start(dummy_s[:, :], zrow[:])
        nc.gpsimd.collective_compute(
            "AllToAll",
            mybir.AluOpType.bypass,
            replica_groups=groups,
            ins=[dummy_s],
            outs=[dummy_r],
        )

    # ---- loads: two balanced issue queues, pieces in consumption order --
    # mm1 seg0 (cols 0-256) needs x[:, :, 0:256] plus ALL of w1 (16 f-tiles)
    # first; w2 is consumed f-ascending starting at the first mm2 tile.
    #   sync (SP):  x cols 0:256 -> w1 f-half2 -> x 256:512 -> x 512:end
    #   scalar:     w1 f-half1 -> w2 (f-ascending quarters)
    #   gpsimd:     combine-weight / scatter-row / gather-idx smalls
    w1sb = wpool.tile([P, DM // P, DF], dt.bfloat16, tag="w1sb")
    w1r = w1t.rearrange("(o p) f -> p o f", p=P)
    xsb = wpool.tile([P, DM // P, ctok], dt.bfloat16, tag="xsb")
    xr = xT.rearrange("(o p) t -> p o t", p=P)
    nc.sync.dma_start(xsb[:, :, 0:256], xr[:, :, 0:256])
    for k0 in range(0, DM // P, 4):
        nc.scalar.dma_start(
            w1sb[:, k0 : k0 + 4, 0 : DF // 2], w1r[:, k0 : k0 + 4, 0 : DF // 2]
        )
        nc.sync.dma_start(
            w1sb[:, k0 : k0 + 4, DF // 2 : DF], w1r[:, k0 : k0 + 4, DF // 2 : DF]
        )
    if ctok > 256:
        nc.sync.dma_start(xsb[:, :, 256:512], xr[:, :, 256:512])
    if ctok > 512:
        nc.sync.dma_start(xsb[:, :, 512:ctok], xr[:, :, 512:ctok])
    w2sb = wpool.tile([P, DF // P, DM], dt.bfloat16, tag="w2sb")
    w2r = w2t.rearrange("(o p) d -> p o d", p=P)
    for f0 in range(0, DF // P, 4):
        nc.scalar.dma_start(w2sb[:, f0 : f0 + 4, :], w2r[:, f0 : f0 + 4, :])
    csb = wpool.tile([P, ntiles], dt.float32, tag="csb")
    nc.gpsimd.dma_start(csb[:], cv.rearrange("(t p) -> p t", p=P))
    rssbs = []
    for g in range(nch):
        rssb = wpool.tile([P, ntiles], dt.int32, tag=f"rssb{g}", name=f"rssb{g}")
        nc.gpsimd.dma_start(rssb[:], rscs[g].rearrange("(t p) -> p t", p=P))
        rssbs.append(rssb)
    g0sb = wpool.tile([CB, OWN // CB], dt.int32, tag="g0sb")
    nc.gpsimd.dma_start(g0sb[:], g0.rearrange("(t p) -> p t", p=CB))
    g1sb = wpool.tile([CB, OWN // CB], dt.int32, tag="g1sb")
    nc.gpsimd.dma_start(g1sb[:], g1.rearrange("(t p) -> p t", p=CB))

    # ---- interleaved mm1 (2-tile segments) / mm2 (token tiles) ----------
    # mm1 segments are 256 cols: wide enough that ldweights stays hidden
    # under the moving dim, narrow enough that mm2 tiles start early
    segs = []
    t = 0
    while t < ntiles:
        b = min(t + 2, ntiles)
        segs.append((t, b))
        t = b

    fired = [False] * nch
    scat_gate = [None] * nch
    hs = None
    hbase = 0
    done_mm1 = 0
    si = 0

    def emit_mm1():
        nonlocal hs, hbase, done_mm1, si
        a, b = segs[si]
        si += 1
        c0, csz = a * P, (b - a) * P
        # skip the padding columns beyond the real token count: mm2 reads
        # the stale hs region for pad tokens, whose y rows scatter to the
        # garbage row anyway
        csz_r = min(csz, tr - c0)
        hbase = c0
        done_mm1 = c0 + csz
        hs = [
            hpool.tile([P, csz], dt.bfloat16, tag=f"h{i}", name=f"h{i}")
            for i in range(DF // P)
        ]
        for i in range(DF // P):
            ph = phpool.tile([P, csz_r], dt.float32, tag="ph")
            for k in range(DM // P):
                nc.tensor.matmul(
                    ph[:],
                    lhsT=w1sb[:, k, i * P : (i + 1) * P],
                    rhs=xsb[:, k, c0 : c0 + csz_r],
                    start=(k == 0),
                    stop=(k == DM // P - 1),
                )
            nc.scalar.activation(hs[i][:, 0:csz_r], ph[:], ACT)

    for tm in range(ntiles):
        while (tm + 1) * P > done_mm1:
            emit_mm1()
        toff = tm * P - hbase
        py0 = pypool.tile([P, 512], dt.float32, tag="py")
        py1 = pypool.tile([P, 512], dt.float32, tag="py")
        for f in range(DF // P):
            lhs = hs[f][:, toff : toff + P]
            nc.tensor.matmul(
                py0[:], lhsT=lhs, rhs=w2sb[:, f, 0:512],
                start=(f == 0), stop=(f == DF // P - 1),
            )
            nc.tensor.matmul(
                py1[:], lhsT=lhs, rhs=w2sb[:, f, 512:1024],
                start=(f == 0), stop=(f == DF // P - 1),
            )
        # scale on the (otherwise idle) Scalar engine: keeping these off the
        # Vector queue stops the combine adds' gather-waits from head-blocking
        # the PSUM release chain
        y_sb = ypool.tile([P, DM], dt.bfloat16, tag="y")
        nc.scalar.activation(
            y_sb[:, 0:512], py0[:],
            mybir.ActivationFunctionType.Copy, scale=csb[:, tm : tm + 1],
        )
        nc.scalar.activation(
            y_sb[:, 512:1024], py1[:],
            mybir.ActivationFunctionType.Copy, scale=csb[:, tm : tm + 1],
        )
        for g in gsets[tm]:
            last_scat = nc.gpsimd.indirect_dma_start(
                out=sends[g][:],
                out_offset=bass.IndirectOffsetOnAxis(
                    ap=rssbs[g][:, tm : tm + 1], axis=0
                ),
                in_=y_sb[:],
                in_offset=None,
            )
        for g in range(nch):
            if not fired[g] and Ts[g] == tm + 1:
                nc.gpsimd.collective_compute(
                    "AllToAll",
                    mybir.AluOpType.bypass,
                    replica_groups=groups,
                    ins=[sends[g][0 : NCORES * sg[g], :]],
                    outs=[recvbuf[Rg[g] : Rg[g + 1], :]],
                )
                fired[g] = True
                scat_gate[g] = last_scat
    assert all(fired), (Ts, ntiles)

    # ---- combine: class-sorted 64-token tiles, prefix-sliced gathers ----
    # per-token rows are host-ordered (g0 = earlier-landing chunk, g1 =
    # later), each gather gated/prefixed by its own class so the early-row
    # gather overlaps the last A2A chunk instead of waiting for it
    for j in range(OWN // CB):
        ga = gpool.tile([CB, DM], dt.bfloat16, tag="ga")
        gi0 = nc.gpsimd.indirect_dma_start(
            out=ga[:],
            out_offset=None,
            in_=recvbuf[0 : Rg[comb_cls0[j] + 1], :],
            in_offset=bass.IndirectOffsetOnAxis(ap=g0sb[:, j : j + 1], axis=0),
        )
        gb = gpool.tile([CB, DM], dt.bfloat16, tag="gb")
        gi1 = nc.gpsimd.indirect_dma_start(
            out=gb[:],
            out_offset=None,
            in_=recvbuf[0 : Rg[comb_cls1[j] + 1], :],
            in_offset=bass.IndirectOffsetOnAxis(ap=g1sb[:, j : j + 1], axis=0),
        )
        # queue class-c gathers behind the scatter that fires chunk c+1: the
        # scheduler's optimistic A2A cost model otherwise slots them between
        # scatters where their (blocked) recvbuf wait head-blocks the gpsimd
        # queue and delays the next A2A trigger. Gating on c+1's scatter
        # (not the overall last) still lets early-class gathers overlap the
        # later A2A chunks.
        for gi, c in ((gi0, comb_cls0[j]), (gi1, comb_cls1[j])):
            gate = scat_gate[c + 1] if c + 1 < nch else scat_gate[nch - 1]
            add_dep_helper(gi.ins, gate.ins, sync=False, reason="scatters first")
        ys = gpool.tile([CB, DM], dt.float32, tag="ys")
        nc.vector.tensor_add(ys[:], ga[:], gb[:])
        nc.sync.dma_start(yout[j * CB : (j + 1) * CB, :], ys[:])


def _build_program(cfg):
    key = cfg[:8]
    if key in _PROGRAM_CACHE:
        return _PROGRAM_CACHE[key]
    from contextlib import ExitStack

    nc = bacc.Bacc(
        "TRN2",
        target_bir_lowering=False,
        debug=False,
        enable_asserts=True,
        num_devices=NCORES,
    )
    with tile.TileContext(nc) as tc:
        with ExitStack() as ctx:
            _emit(nc, tc, ctx, cfg)
    nc.compile()
    _PROGRAM_CACHE[key] = nc
    return nc


def _prepare(x, topk_e, topk_w):
    """Host-side routing: dispatch tokens to experts.

    Layout: token with slot s in (expert e -> owner d) block, s in
    [B_g, B_{g+1}):  send-side row (core e, tensor send_g) = d*sg + (s-B_g);
    recv-side row (core d, recvbuf) = 8*B_g + e*sg + (s-B_g).
    """
    bf16 = ml_dtypes.bfloat16
    c = np.zeros((S, E), dtype=np.float32)
    np.add.at(c, (np.arange(S)[:, None], topk_e), topk_w.astype(np.float32))
    single = topk_e[:, 0] == topk_e[:, 1]

    toks = [np.nonzero((topk_e == e).any(axis=1))[0] for e in range(E)]
    cnt = np.zeros((E, NCORES), dtype=np.int64)
    for e in range(E):
        cnt[e] = np.bincount(toks[e] // OWN, minlength=NCORES)
    blk = int(cnt.max())

    ntok_max = max(len(t) for t in toks)
    ctok = int(-(-ntok_max // P) * P)
    ntiles = ctok // P

    # Adaptive chunk boundaries: chunk g should trigger after tile
    # targets[g], i.e. the largest 16-multiple slot b such that every
    # expert's token count with slot < b fits in targets[g]*128 rows.
    # Late triggers get small chunks so the post-compute tail is short.
    def maxcum(b):
        return int(np.minimum(cnt, b).sum(axis=1).max())

    targets = [ntiles // 2, ntiles - 2, ntiles - 1]
    B = [0]
    for tgt in targets:
        b = B[-1]
        nb = b
        while nb + 16 < blk and maxcum(nb + 16) <= tgt * P:
            nb += 16
        if nb > b:
            B.append(nb)
    B.append(blk)
    B = sorted(set(B))
    nch = len(B) - 1
    Ba = np.array(B)
    sga = np.diff(Ba)
    Rga = NCORES * Ba

    in_maps = []
    row_of = {}  # (e, token) -> absolute recvbuf row (on the owner core)
    cums = np.zeros((E, nch), dtype=np.int64)
    gsets = [set() for _ in range(ntiles)]
    for e in range(E):
        te = toks[e]
        d = te // OWN
        seg_start = np.searchsorted(te, np.arange(NCORES) * OWN)
        slot = np.arange(len(te)) - seg_start[d]
        gi = np.searchsorted(Ba[1:-1], slot, side="right")
        srow = d * sga[gi] + (slot - Ba[gi])  # send side, relative to send_g
        rrow = Rga[gi] + e * sga[gi] + (slot - Ba[gi])  # recv side, absolute
        for t, r in zip(te, rrow):
            row_of[(e, int(t))] = int(r)
        order = np.lexsort((slot, d, gi))
        te_o = te[order]
        gi_o = np.full(ctok, nch - 1, dtype=np.int64)
        gi_o[: len(te)] = gi[order]
        xT_e = np.zeros((DM, ctok), dtype=bf16)
        xT_e[:, : len(te)] = x[te_o].T.astype(bf16)
        cv_e = np.zeros(ctok, dtype=np.float32)
        w = c[te_o, e]
        cv_e[: len(te)] = np.where(single[te_o], 0.5 * w, w)
        im = {"xT": xT_e, "cv": cv_e}
        srow_o = np.zeros(ctok, dtype=np.int64)
        srow_o[: len(te)] = srow[order]
        for g in range(nch):
            rs = np.full(ctok, NCORES * sga[g], dtype=np.int32)  # garbage row
            sel = gi_o == g
            sel[len(te) :] = False
            rs[sel] = srow_o[sel]
            im[f"rsc{g}"] = rs
            cums[e, g] = int(np.sum(gi <= g))
        for tm in range(ntiles):
            for g in np.unique(gi_o[tm * P : (tm + 1) * P]):
                gsets[tm].add(int(g))
        in_maps.append(im)

    Ts = [min(int(np.ceil(cums[:, g].max() / P)), ntiles) for g in range(nch)]
    for g in range(1, nch):
        Ts[g] = max(Ts[g], Ts[g - 1])
    Ts[-1] = ntiles

    chunk_of_row = lambda r: int(np.searchsorted(Rga[1:], r, side="right"))
    perms = []
    comb_cls0 = np.zeros((NCORES, OWN // CB), dtype=np.int64)
    comb_cls1 = np.zeros((NCORES, OWN // CB), dtype=np.int64)
    for dcore in range(NCORES):
        r0a = np.zeros(OWN, dtype=np.int32)
        r1a = np.zeros(OWN, dtype=np.int32)
        cls0 = np.zeros(OWN, dtype=np.int64)
        cls1 = np.zeros(OWN, dtype=np.int64)
        for t_loc in range(OWN):
            t = dcore * OWN + t_loc
            es = np.unique(topk_e[t])
            ra = row_of[(int(es[0]), t)]
            rb = row_of[(int(es[1]), t)] if len(es) > 1 else ra
            ca, cb = chunk_of_row(ra), chunk_of_row(rb)
            if ca > cb:
                ra, rb, ca, cb = rb, ra, cb, ca
            r0a[t_loc], r1a[t_loc] = ra, rb
            cls0[t_loc], cls1[t_loc] = ca, cb
        perm = np.argsort(cls1, kind="stable")
        perms.append(perm)
        comb_cls0[dcore] = cls0[perm].reshape(OWN // CB, CB).max(axis=1)
        comb_cls1[dcore] = cls1[perm].reshape(OWN // CB, CB).max(axis=1)
        in_maps[dcore]["g0"] = r0a[perm]
        in_maps[dcore]["g1"] = r1a[perm]

    cfg = (
        blk,
        tuple(B),
        ctok,
        ntok_max,
        tuple(Ts),
        tuple(int(v) for v in comb_cls0.max(axis=0)),
        tuple(int(v) for v in comb_cls1.max(axis=0)),
        tuple(tuple(sorted(s)) for s in gsets),
        tuple(tuple(int(v) for v in p) for p in perms),
    )
    return in_maps, cfg


def prepare_in_maps(x, topk_e, topk_w, w1, w2):
    bf16 = ml_dtypes.bfloat16
    in_maps, cfg = _prepare(np.asarray(x), np.asarray(topk_e), np.asarray(topk_w))
    for e in range(E):
        in_maps[e]["w1t"] = np.ascontiguousarray(np.asarray(w1)[e].T).astype(bf16)
        in_maps[e]["w2t"] = np.ascontiguousarray(np.asarray(w2)[e].T).astype(bf16)
    return in_maps, cfg


def postprocess(results, cfg):
    perms = cfg[8]
    out = np.empty((S, DM), dtype=np.float32)
    for d in range(NCORES):
        out[d * OWN + np.asarray(perms[d], dtype=np.int64)] = results[d]["yout"]
    return out


def kernel(x, topk_e, topk_w, w1, w2):
    in_maps, cfg = prepare_in_maps(x, topk_e, topk_w, w1, w2)
    nc = _build_program(cfg)
    res = run_bass_kernel_spmd(nc, in_maps, list(range(NCORES)))
    return postprocess(res.results, cfg)
